# revision 113
# baseline (speedup 1.0000x reference)
"""Trainium2 Bass kernel for nn_Attention_66907000537586.

Module: x -> 1x1conv+BN (Q,K,V) -> 8-head attention with relative position
bias -> exact GELU -> 1x1conv+bias+BN.  Shapes: B=8, C=256, F=32 (n=1024
tokens), H=8, DK=32, DV=64.

Sharding: pure data-parallel over batch (one batch element per NeuronCore,
8 cores), no collectives.  BN/scale folding happens on host; x is cast to
bf16 on host.  The relative position bias is block-Toeplitz; the host builds
a compact *shifted* table of exp(bias) so a single 2D SBUF slice yields any
(128 x 1024) transposed-bias tile, and softmax uses exp(dots+bias) =
exp(dots) * exp(bias).

Key structure per core (batch element b):
  - Q/K projections produce the 4x row-group-replicated per-head layout
    DIRECTLY (folded weight columns replicated host-side, per head q|k
    interleaved), so no replication DMAs are needed.  Heads 0-1 project up
    front; head h+2's projection is emitted inside head h's pass-1 and hides
    under the attention pipeline.
  - V^T (1024 x 8*(64+1)) carries a ones column per head so the attention
    matmul also produces the softmax denominator S; the per-channel BN
    offsets ride into PSUM via a 1-row ones x voffi matmul, making the
    evacuation a plain copy that alternates ScalarE/VectorE.
  - per head h, per j-tile: dotsT = K_h^T Q_h (K=32 contraction rotating all
    four 32-row PE tile_position groups), et = exp(dotsT) on ScalarE,
    et2 = et * expbias-slice split between VectorE and the Pool engine.
  - normalization: S row -> DRAM bounce -> (128,8) gather -> reciprocal ->
    scatter -> (64,N) broadcast read; the multiply for head h is emitted
    inside head h+1's pass-1, so the whole ~8us DMA latency hides under the
    dots/exp pipeline of the next head (no exposed stall except the last
    head's, which overlaps the output conv).
  - all exact GELUs batch at the very end (one act-table switch); the
    output conv's k=3 contraction block is split by head so only head 7's
    64 rows wait on the last norm chain; output is stored as bf16 (the
    host returns fp32) with stores split across queues.
"""

import numpy as np
import ml_dtypes

HEADS, DK, DV, F = 8, 32, 64, 32
C = 256
N = F * F            # 1024 tokens
B = 8
EPS = 1e-5
IDK = HEADS * DK     # 256
IDV = HEADS * DV     # 512
VTW = HEADS * (DV + 1)   # 520
SW = 2112            # per-head width of the shifted compact bias table
NJT = N // 128       # 8 j-tiles
NIT = N // 512       # 2 i-tiles

_PROGRAM_CACHE = {}

# which j-tiles' bias multiplies ride the Pool engine (it is ~3x slower per
# op than VectorE, but has plenty of slack before pass-2 consumes them)
GPSIMD_JS = (4,)


def _split_excess_waits(nc, mybir, limit=1):
    """Two post-passes over the scheduled BIR:

    1. Drop PE->PE self-semaphore waits from PE instructions.  TensorE
       matmuls complete strictly in program order, and every PSUM-slot
       reuse in this kernel is already guarded by the consumer engine's
       wait (ScalarE/VectorE read the slot before it rotates), so the
       self-wait is redundant -- and it forces each matmul to wait for
       the *completion* (drain) of in-flight matmuls, which defeats
       tile_position row-group concurrency entirely.

    2. The walrus build in this container rejects instructions carrying
       more than `limit` semaphore sync-waits.  Move the excess onto
       carrier NoOps inserted just before, on the same engine (same
       queue => same ordering)."""
    k = 0
    for fn in nc.m.functions:
        for bb in fn.blocks:
            out = []
            for inst in bb.instructions:
                si = inst.sync_info
                if (si is not None and si.on_wait
                        and str(inst.engine) == "EngineType.PE"
                        and type(inst).__name__ in ("InstMatmult", "InstLdweights")):
                    kept = [w for w in si.on_wait
                            if not str(w.ant_name).startswith("PE_")]
                    if len(kept) != len(si.on_wait):
                        si.on_wait = kept
                waits = list(si.on_wait) if si is not None else []
                if len(waits) > limit:
                    extra, keep = waits[:-limit], waits[-limit:]
                    for i in range(0, len(extra), limit):
                        nop = mybir.InstNoOp(name=f"waitsplit_{k}")
                        k += 1
                        nop.engine = inst.engine
                        nop.sync_info = mybir.SyncInfo(
                            on_wait=extra[i:i + limit], on_update=[])
                        out.append(nop)
                    si.on_wait = keep
                out.append(inst)
            bb.instructions = out


def build_program(structured=True):
    """Build the single-core Bass program (run SPMD on 8 cores)."""
    import concourse.bass as bass
    import concourse.mybir as mybir
    import concourse.tile as tile

    dt = mybir.dt
    nc = bass.Bass("TRN2", target_bir_lowering=False, debug=False, num_devices=B)

    f32, bf16 = dt.float32, dt.bfloat16

    xb = nc.dram_tensor("xb", [C, N], bf16, kind="ExternalInput")
    wqk = nc.dram_tensor("wqk", [C, HEADS * 256], bf16, kind="ExternalInput")
    qkoff = nc.dram_tensor("qkoff", [128, 16], f32, kind="ExternalInput")
    wvt = nc.dram_tensor("wvt", [C, VTW], bf16, kind="ExternalInput")
    voffi = nc.dram_tensor("voffi", [1, VTW], bf16, kind="ExternalInput")
    wot = nc.dram_tensor("wot", [IDV, C], bf16, kind="ExternalInput")
    ooff = nc.dram_tensor("ooff", [128, 2], f32, kind="ExternalInput")
    if structured:
        sst = nc.dram_tensor("sst", [128, HEADS * SW], bf16, kind="ExternalInput")
    else:
        sst = nc.dram_tensor("sst", [HEADS * NJT * 128, N], bf16, kind="ExternalInput")
    out = nc.dram_tensor("out", [C, N], bf16, kind="ExternalOutput")

    ident_dram = None
    if not structured:
        ident_np = np.eye(128, dtype=ml_dtypes.bfloat16)
        ident_dram = nc.inline_tensor(ident_np, name="ident128")

    ExpF = mybir.ActivationFunctionType.Exp
    GeluF = mybir.ActivationFunctionType.Gelu
    IdF = mybir.ActivationFunctionType.Identity

    with tile.TileContext(nc) as tc:
        with (
            tc.tile_pool(name="persist", bufs=1) as pp,
            tc.tile_pool(name="exps", bufs=3) as ep,
            tc.tile_pool(name="exps2", bufs=12) as e2p,
            tc.tile_pool(name="norm", bufs=2) as np_pool,
            tc.tile_pool(name="bias_stream", bufs=3) as bp,
            tc.tile_pool(name="dramscratch", bufs=2, space="DRAM") as dp,
            tc.tile_pool(name="ps8", bufs=2, space="PSUM") as ps8,
        ):
            # ---- constants ----
            ones = pp.tile([1, 128], bf16, tag="ones")
            nc.vector.memset(ones, 1.0)

            # ---- startup DMAs, ordered for earliest first matmul ----
            # sync: wqk (head-pair chunks in head order) + qkoff then the
            # bias table; scalar: x.
            wqksb = [pp.tile([128, HEADS * 256], bf16, tag=f"wqk{k}",
                             name=f"wqk{k}")
                     for k in range(2)]
            for g in range(4):
                for k in range(2):
                    nc.sync.dma_start(
                        out=wqksb[k][:, g * 512:(g + 1) * 512],
                        in_=wqk.ap()[k * 128:(k + 1) * 128,
                                     g * 512:(g + 1) * 512])
            qkoffsb = pp.tile([128, 16], f32, tag="qkoff")
            nc.sync.dma_start(out=qkoffsb, in_=qkoff.ap())
            xbf = [pp.tile([128, N], bf16, tag=f"xbf{k}", name=f"xbf{k}")
                   for k in range(2)]
            for k in range(2):
                for ph in range(2):
                    nc.scalar.dma_start(
                        out=xbf[k][64 * ph:64 * (ph + 1), :],
                        in_=xb.ap()[k * 128 + 64 * ph:k * 128 + 64 * (ph + 1), :])

            # gpsimd (SWDGE): V weights.
            voffisb = pp.tile([1, VTW], bf16, tag="voffi")
            nc.gpsimd.dma_start(out=voffisb, in_=voffi.ap())
            wvtsb = [pp.tile([128, VTW], bf16, tag=f"wvt{k}", name=f"wvt{k}")
                     for k in range(2)]
            last_wvt = None
            for k in range(2):
                for ph in range(2):
                    last_wvt = nc.gpsimd.dma_start(
                        out=wvtsb[k][64 * ph:64 * (ph + 1), :],
                        in_=wvt.ap()[k * 128 + 64 * ph:k * 128 + 64 * (ph + 1), :])

            sstsb = None
            if structured:
                from concourse.tile import add_dep_helper
                sstsb = pp.tile([128, HEADS * SW], bf16, tag="sst")
                # the 4.2MB bias table has huge slack (head h's slice isn't
                # needed until ~9us per head in); hard-gate its first chunks
                # on the projection inputs' completion so x/wqk/wvt get the
                # full HBM bandwidth, then stream in head order.
                first = [None, None]
                for h in range(HEADS):
                    eng = nc.gpsimd if h < HEADS // 2 else nc.sync
                    for ph in range(2):
                        d = eng.dma_start(
                            out=sstsb[64 * ph:64 * (ph + 1), h * SW:(h + 1) * SW],
                            in_=sst.ap()[64 * ph:64 * (ph + 1),
                                         h * SW:(h + 1) * SW],
                        )
                        idx = 0 if h < HEADS // 2 else 1
                        if first[idx] is None:
                            first[idx] = d
                            add_dep_helper(d.ins, last_wvt.ins, sync=True,
                                           reason="sst after proj inputs")

            identsb = None
            if not structured:
                identsb = pp.tile([128, 128], bf16, tag="ident")
                nc.sync.dma_start(out=identsb, in_=ident_dram.ap())

            # ---- projections (4x row-group-replicated per head) ----
            qsb = [pp.tile([128, N], bf16, tag=f"qsb{h}", name=f"qsb{h}")
                   for h in range(HEADS)]
            ksb = [pp.tile([128, N], bf16, tag=f"ksb{h}", name=f"ksb{h}")
                   for h in range(HEADS)]
            vtsb = [pp.tile([128, VTW], bf16, tag=f"vtsb{j}", name=f"vtsb{j}")
                    for j in range(NJT)]

            def emit_qk_one(h, idx, engine="vector"):
                # idx 0 = q, 1 = k.  PSUM rides the "ou" ring: its slots are
                # free mid-pass-1 (the OU output is evacuated to SBUF right
                # after accumulation), so this never disturbs the dts ring.
                dst, colbase, ocol = (
                    (qsb, 256 * h, 2 * h) if idx == 0
                    else (ksb, 256 * h + 128, 2 * h + 1))
                ps = ps8.tile([128, N], f32, tag="ou", bufs=2, name="qkps")
                for nt in range(NIT):
                    for k in range(2):
                        nc.tensor.matmul(
                            ps[:, nt * 512:(nt + 1) * 512],
                            lhsT=wqksb[k][:, colbase:colbase + 128],
                            rhs=xbf[k][:, nt * 512:(nt + 1) * 512],
                            start=(k == 0), stop=(k == 1),
                        )
                if engine == "scalar":
                    nc.scalar.activation(dst[h], ps, IdF,
                                         bias=qkoffsb[:, ocol:ocol + 1])
                else:
                    nc.vector.tensor_scalar_add(dst[h], ps,
                                                qkoffsb[:, ocol:ocol + 1])

            def emit_v(j, engine="vector", tag="ps"):
                # the broadcast BN offsets (+ ones cols) ride into PSUM via
                # a 1-row ones x voffi matmul, so evacuation is a plain copy.
                ps = ps8.tile([128, VTW], f32, tag=tag, bufs=2, name="vps")
                for (lo, hi) in ((0, 512), (512, VTW)):
                    for k in range(2):
                        nc.tensor.matmul(
                            ps[:, lo:hi],
                            lhsT=xbf[k][:, j * 128:(j + 1) * 128],
                            rhs=wvtsb[k][:, lo:hi],
                            start=(k == 0), stop=False,
                        )
                    nc.tensor.matmul(ps[:, lo:hi], lhsT=ones,
                                     rhs=voffisb[:, lo:hi],
                                     start=False, stop=True)
                if engine == "scalar":
                    nc.scalar.copy(vtsb[j], ps)
                else:
                    nc.vector.tensor_copy(vtsb[j], ps)

            # Q/K for heads 0-1 and V^T tiles 0-1 up front; V 2-7 interleave
            # into head 0's pass-1 (they're only needed by its OU burst) and
            # Q/K for heads 2-7 stream in during the first six heads.
            emit_qk_one(0, 0, "scalar")
            emit_qk_one(0, 1, "vector")
            emit_qk_one(1, 0, "scalar")
            emit_qk_one(1, 1, "vector")
            emit_v(0, "scalar", "ps")
            emit_v(1, "vector", "ps")

            # ---- attention ----
            gsb = [pp.tile([128, N], bf16, tag=f"gsb{t}", name=f"gsb{t}")
                   for t in range(4)]

            def start_norm(h, osb):
                """Emit the S -> 1/S broadcast DMA chain for head h (latency
                hides under head h+1's pass-1)."""
                rd = dp.tile([1, N], bf16, tag="rd", name="rd")
                nc.sync.dma_start(out=rd, in_=osb[64:65, :])
                s8 = np_pool.tile([128, 8], bf16, tag="s8", name="s8")
                nc.sync.dma_start(
                    out=s8,
                    in_=bass.AP(tensor=rd.tensor, offset=rd.offset,
                                ap=[[8, 128], [1, 8]]),
                )
                r8 = np_pool.tile([128, 8], f32, tag="r8", name="r8")
                nc.vector.reciprocal(r8, s8)
                r8b = np_pool.tile([128, 8], bf16, tag="r8b", name="r8b")
                nc.vector.tensor_copy(r8b, r8)
                rd2 = dp.tile([1, N], bf16, tag="rd2", name="rd2")
                nc.sync.dma_start(
                    out=bass.AP(tensor=rd2.tensor, offset=rd2.offset,
                                ap=[[8, 128], [1, 8]]),
                    in_=r8b,
                )
                # bf16 broadcast: halves the DMA and enables the 2x-rate
                # bf16 multiply on VectorE.
                rb = np_pool.tile([64, N], bf16, tag="rb", name="rb")
                nc.sync.dma_start(
                    out=rb,
                    in_=bass.AP(tensor=rd2.tensor, offset=rd2.offset,
                                ap=[[0, 64], [1, N]]),
                )
                return (h, osb, rb)

            def finish_norm(state):
                # gelus batch at the very end (one act-table switch): after
                # norm(6) everything except gsb[3]'s head-7 half is ready,
                # so the conv can run k=0..2 and half of k=3 under head 7's
                # norm-chain latency.
                hp, osb, rb = state
                nc.vector.tensor_mul(
                    gsb[hp // 2][64 * (hp % 2):64 * (hp % 2) + 64, :],
                    osb[0:64, :], rb)
                if hp == HEADS - 2:
                    for t in (0, 1, 2):
                        nc.scalar.activation(gsb[t], gsb[t], GeluF)
                    nc.scalar.activation(gsb[3][0:64, :], gsb[3][0:64, :],
                                         GeluF)
                if hp == HEADS - 1:
                    nc.scalar.activation(gsb[3][64:128, :], gsb[3][64:128, :],
                                         GeluF)

            def emit_dots(h, j):
                """dots -> exp -> bias-multiply for one (head, j-tile)."""
                dts = ps8.tile([128, N], f32, tag="ps", name="dots")
                if not structured:
                    bt = bp.tile([128, N], bf16, tag="bt", name="bt")
                    base = (h * NJT + j) * 128
                    nc.sync.dma_start(out=bt, in_=sst.ap()[base:base + 128, :])
                for it in range(NIT):
                    sl = slice(it * 512, (it + 1) * 512)
                    rg = 32 * ((2 * j + it) % 4)  # rotate all 4 PE row groups
                    nc.tensor.matmul(
                        dts[:, sl],
                        lhsT=ksb[h][rg:rg + 32, j * 128:(j + 1) * 128],
                        rhs=qsb[h][rg:rg + 32, sl],
                        start=True, stop=structured,
                        tile_position=(rg, 0),
                    )
                    if not structured:
                        nc.tensor.matmul(dts[:, sl], lhsT=identsb,
                                         rhs=bt[:, sl],
                                         start=False, stop=True)
                et = ep.tile([128, N], bf16, tag="et", name="et")
                nc.scalar.activation(et, dts, ExpF)
                if structured:
                    off = h * SW + (31 - 4 * j) * 32
                    et2 = e2p.tile([128, N], bf16, tag="et2", name="et2")
                    eng = nc.gpsimd if j in GPSIMD_JS else nc.vector
                    eng.tensor_mul(et2, et, sstsb[:, off:off + N])
                else:
                    et2 = et
                return et2

            def emit_ou(h, ets, j):
                for it in range(NIT):
                    sl = slice(it * 512, (it + 1) * 512)
                    nc.tensor.matmul(
                        ous_live[h][:, sl],
                        lhsT=vtsb[j][:, h * 65:h * 65 + 65],
                        rhs=ets[j][:, sl],
                        start=(j == 0), stop=(j == NJT - 1),
                    )

            # Software-pipelined head loop: iteration h emits pass-1 tiles
            # j=4..7 of head h (j=0..3 were emitted interleaved with head
            # h-1's OU burst), head h+2's projection at the slack points,
            # then the lead-in dots of head h+1 round-robined with head h's
            # OU accumulation so the ScalarE exp chain never starves.
            ets_by_head = {0: {}}
            ous_live = {}
            pending = None
            for h in range(HEADS):
                if h == 1:
                    # output-conv weights; issued on the now-idle sync queue,
                    # needed only from ~h==4 on.
                    wotsb = [pp.tile([128, C], bf16, tag=f"wot{k}", name=f"wot{k}")
                             for k in range(4)]
                    for k in range(4):
                        nc.sync.dma_start(out=wotsb[k],
                                          in_=wot.ap()[k * 128:(k + 1) * 128, :])
                    ooffsb = pp.tile([128, 2], f32, tag="ooff")
                    nc.sync.dma_start(out=ooffsb, in_=ooff.ap())
                ets = ets_by_head[h]
                jstart = 0 if h == 0 else 4
                if h == HEADS - 1:
                    # last head: its j=0..3 et2 tiles already exist (lead-in),
                    # so those OU matmuls interleave into pass-1 and the
                    # exposed norm chain starts ~2us earlier.
                    ous_live[h] = ps8.tile([65, N], f32, tag="ou", bufs=2,
                                           name=f"ou{h}")
                for j in range(jstart, NJT):
                    ets[j] = emit_dots(h, j)
                    if h == HEADS - 1 and j in (4, 5):
                        emit_ou(h, ets, 2 * (j - 4))
                        emit_ou(h, ets, 2 * (j - 4) + 1)
                    if h == 0 and j < 6:
                        # V^T tiles 2-7 ride head 0's exp-paced pass-1
                        emit_v(j + 2, "vector", "ou")
                    if j == 4 and h + 2 < HEADS:
                        emit_qk_one(h + 2, 0)
                    if j == 6 and h + 2 < HEADS:
                        emit_qk_one(h + 2, 1)
                # pass 2 (+ next head's lead-in): rows 0-63 the output, row
                # 64 the softmax denominator S.
                if h != HEADS - 1:
                    ous_live[h] = ps8.tile([65, N], f32, tag="ou", bufs=2,
                                           name=f"ou{h}")
                if h + 1 < HEADS:
                    ets_by_head[h + 1] = {}
                    for jn in range(4):
                        ets_by_head[h + 1][jn] = emit_dots(h + 1, jn)
                        if jn == 1 and pending is not None:
                            finish_norm(pending)
                            pending = None
                        emit_ou(h, ets, 2 * jn)
                        emit_ou(h, ets, 2 * jn + 1)
                else:
                    if pending is not None:
                        finish_norm(pending)
                        pending = None
                    for j in range(4, NJT):
                        emit_ou(h, ets, j)
                # evacuate OU (incl. the S row) to SBUF in bf16 immediately:
                # frees the PSUM slot and decouples the norm-chain latency
                # from the PSUM ring.
                osb = e2p.tile([65, N], bf16, tag="osb", bufs=2,
                               name=f"osb{h}")
                nc.vector.tensor_copy(osb, ous_live[h])
                del ets_by_head[h]
                pending = start_norm(h, osb)

            finish_norm(pending)

            # ---- output conv; contraction split (k3a = head 6 rows, k3b =
            # head 7 rows) so only k3b waits on the last head's norm chain.
            convps = {m: ps8.tile([128, N], f32, tag="ps", name=f"cps{m}")
                      for m in range(2)}
            chunks = ([(k, 0, 128, k == 0, False) for k in range(3)]
                      + [(3, 0, 64, False, False), (3, 64, 128, False, True)])
            for (k, r0, r1, st, sp) in chunks:
                for m in range(2):
                    for nt in range(NIT):
                        sl = slice(nt * 512, (nt + 1) * 512)
                        nc.tensor.matmul(
                            convps[m][:, sl],
                            lhsT=wotsb[k][r0:r1, m * 128:(m + 1) * 128],
                            rhs=gsb[k][r0:r1, sl],
                            start=st, stop=sp,
                        )
            for m in range(2):
                ysb = np_pool.tile([128, N], bf16, tag="ysb", name="ysb")
                for nt in range(NIT):
                    sl = slice(nt * 512, (nt + 1) * 512)
                    nc.vector.tensor_scalar_add(ysb[:, sl], convps[m][:, sl],
                                                ooffsb[:, m:m + 1])
                    for ph in range(2):  # split store issue across engines
                        eng = nc.sync if ph == 0 else nc.scalar
                        eng.dma_start(
                            out=out.ap()[m * 128 + 64 * ph:m * 128 + 64 * (ph + 1),
                                         nt * 512:(nt + 1) * 512],
                            in_=ysb[64 * ph:64 * (ph + 1), sl])

    _split_excess_waits(nc, mybir)
    return nc


def _fold_inputs(inp):
    """Host-side BN/scale folding + compact bias table construction."""
    f32 = np.float32
    bfc = ml_dtypes.bfloat16
    scale = DK ** -0.5
    x = np.asarray(inp["x"], f32)

    def bn_fold(w, gam, bet, mu, var, s=1.0):
        inv = np.asarray(gam, f32) / np.sqrt(np.asarray(var, f32) + EPS)
        wf = (np.asarray(w, f32) * inv[:, None] * s).T.copy()        # (C, O)
        off = (np.asarray(bet, f32) - np.asarray(mu, f32) * inv) * s  # (O,)
        return wf, off

    wqt, qoff = bn_fold(inp["wq"], inp["qgam"], inp["qbet"], inp["qmu"],
                        inp["qvar"], scale)
    wkt, koff = bn_fold(inp["wk"], inp["kgam"], inp["kbet"], inp["kmu"],
                        inp["kvar"])
    wvt0, voff = bn_fold(inp["wv"], inp["vgam"], inp["vbet"], inp["vmu"],
                         inp["vvar"])

    # replicated, per-head interleaved: head h -> [q_rep(128) | k_rep(128)]
    wqk = np.zeros((C, HEADS * 256), f32)
    qkoff_v = np.zeros((128, 16), f32)
    for h in range(HEADS):
        wqk[:, 256 * h:256 * h + 128] = np.tile(wqt[:, 32 * h:32 * h + 32], (1, 4))
        wqk[:, 256 * h + 128:256 * h + 256] = np.tile(wkt[:, 32 * h:32 * h + 32], (1, 4))
        qkoff_v[:, 2 * h] = np.tile(qoff[32 * h:32 * h + 32], 4)
        qkoff_v[:, 2 * h + 1] = np.tile(koff[32 * h:32 * h + 32], 4)

    wvt = np.zeros((C, VTW), f32)
    voffi = np.zeros((1, VTW), f32)
    for h in range(HEADS):
        wvt[:, 65 * h:65 * h + 64] = wvt0[:, 64 * h:64 * h + 64]
        voffi[0, 65 * h:65 * h + 64] = voff[64 * h:64 * h + 64]
        voffi[0, 65 * h + 64] = 1.0   # ones column -> softmax denominator

    oinv = np.asarray(inp["ogam"], f32) / np.sqrt(np.asarray(inp["ovar"], f32) + EPS)
    wot = (np.asarray(inp["wo"], f32) * oinv[:, None]).T.copy()       # (512, 256)
    ooff_v = (np.asarray(inp["bo"], f32) * oinv
              + np.asarray(inp["obet"], f32) - np.asarray(inp["omu"], f32) * oinv)
    ooff = np.stack([ooff_v[:128], ooff_v[128:]], axis=1).copy()

    pe = np.asarray(inp["pos_emb"], f32)             # (1024, 8)
    pidx = np.asarray(inp["pos_indices"])            # (1024, 1024) int32

    r = np.arange(F)
    pos = np.stack(np.meshgrid(r, r, indexing="ij"), axis=-1).reshape(-1, 2)
    rel = np.abs(pos[:, None, :] - pos[None, :, :])
    expected = (rel[..., 0] * F + rel[..., 1]).astype(pidx.dtype)
    structured = bool(np.array_equal(pidx, expected))

    if structured:
        dd = np.arange(63)
        xi_ = np.maximum(dd - 31, 0)
        xj_ = np.maximum(31 - dd, 0)
        yy = np.arange(F)
        I = xi_[:, None] * F + yy[None, :]           # (63, yi)
        J = xj_[:, None] * F + yy[None, :]           # (63, yj)
        idx = pidx[I[:, None, :], J[:, :, None]]     # (dd, yj, yi)
        sb = pe[idx] / scale                         # (dd, yj, yi, H)
        flat = np.ascontiguousarray(sb.transpose(3, 1, 0, 2)).reshape(HEADS, 32, 2016)
        eflat = np.exp(flat)   # exp(dots+bias) = exp(dots) * exp(bias)
        sstv = np.zeros((128, HEADS * SW), f32)
        for g in range(4):
            for h in range(HEADS):
                sstv[32 * g:32 * g + 32,
                     h * SW + 32 * g:h * SW + 32 * g + 2016] = eflat[h]
    else:
        biasT = (pe[pidx] / scale).transpose(2, 1, 0)  # (H, j, i)
        sstv = biasT.reshape(HEADS * NJT * 128, N)
    sstv = sstv.astype(bfc)

    common = dict(wqk=wqk.astype(bfc), qkoff=qkoff_v,
                  wvt=wvt.astype(bfc), voffi=voffi.astype(bfc),
                  wot=wot.astype(bfc), ooff=ooff, sst=sstv)
    in_maps = []
    for b in range(B):
        m = dict(common)
        m["xb"] = np.ascontiguousarray(x[b].reshape(C, N)).astype(bfc)
        in_maps.append(m)
    return in_maps, structured


def run(inputs, trace=False, trace_cores=None):
    in_maps, structured = _fold_inputs(inputs)
    key = ("nc", structured)
    if key not in _PROGRAM_CACHE:
        _PROGRAM_CACHE[key] = build_program(structured)
    nc = _PROGRAM_CACHE[key]
    from concourse.bass_utils import run_bass_kernel_spmd
    res = run_bass_kernel_spmd(
        nc, in_maps, core_ids=list(range(B)), trace=trace, trace_cores=trace_cores
    )
    out = np.stack([np.asarray(res.results[b]["out"], dtype=np.float32)
                    for b in range(B)], axis=0)
    return out.reshape(B, C, F, F), res


def kernel(**inputs):
    out, _ = run(inputs, trace=False)
    return out
